# revision 9
# baseline (speedup 1.0000x reference)
"""MoE all-to-all token dispatcher round-trip (permute -> identity expert ->
weighted unpermute) as a Trainium2 Bass kernel, expert-math folded.

Because the expert boundary is the identity, the dispatch round trip reduces
algebraically to

    restored[t]        = hidden[t] * sum_e probs[t, e]
    tokens_per_expert  = routing_map.sum(axis=0)

which is a pure memory-streaming problem: read 128 MB of hidden states, scale
each token row by its (per-token) prob sum, write 128 MB back.  Tokens are
block-sharded across the 8 NeuronCores (1024 tokens each); each core streams
its 16 MB slice through SBUF in [128, 4096] tiles, multiplying by a
per-partition scalar on the vector engine, and separately reduces its
routing-map slice to partial per-expert counts.

Token -> (partition, slot) layout per core: token t_local = p * A + a
(A = 8 slots), so a [128, 64] probs tile is one contiguous 256 B read per
partition and hidden tile `a` at partition p is the contiguous 16 KB row of
token p*A + a.
"""

import numpy as np

T, H, E, NCORES = 8192, 4096, 8, 8
TLOC = T // NCORES  # tokens per core
P = 128             # SBUF partitions
A = TLOC // P       # token slots per partition

_RUNNER = None


def _build_nc(repeats: int = 1):
    import concourse.bacc as bacc
    import concourse.mybir as mybir
    import concourse.tile as tile

    f32 = mybir.dt.float32
    X = mybir.AxisListType.X

    nc = bacc.Bacc(None)
    hs = nc.dram_tensor("hs", [TLOC, H], f32, kind="ExternalInput")
    probs = nc.dram_tensor("probs", [TLOC, E], f32, kind="ExternalInput")
    routing = nc.dram_tensor("routing", [TLOC, E], f32, kind="ExternalInput")
    out = nc.dram_tensor("out", [TLOC, H], f32, kind="ExternalOutput")
    cnt = nc.dram_tensor("cnt", [P, E], f32, kind="ExternalOutput")

    hs_v = hs.rearrange("(p a) h -> a p h", a=A)
    out_v = out.rearrange("(p a) h -> a p h", a=A)
    probs_v = probs.rearrange("(p a) e -> p (a e)", a=A)
    routing_v = routing.rearrange("(p a) e -> p (a e)", a=A)

    with tile.TileContext(nc) as tc:
        with (
            tc.tile_pool(name="small", bufs=1) as small,
            tc.tile_pool(name="data", bufs=4) as data,
        ):
            for _ in range(repeats):
                p_tile = small.tile([P, A * E], f32, tag="p")
                r_tile = small.tile([P, A * E], f32, tag="r")
                s_pa = small.tile([P, A], f32, tag="s")
                c_tile = small.tile([P, E], f32, tag="c")
                nc.sync.dma_start(out=p_tile[:, :], in_=probs_v)
                nc.sync.dma_start(out=r_tile[:, :], in_=routing_v)
                # s_pa[p, a] = sum_e probs[p*A + a, e]
                nc.vector.reduce_sum(
                    out=s_pa[:, :],
                    in_=p_tile[:, :].rearrange("p (a e) -> p a e", e=E),
                    axis=X,
                )
                # c_tile[p, e] = sum_a routing[p*A + a, e]
                nc.vector.reduce_sum(
                    out=c_tile[:, :],
                    in_=r_tile[:, :].rearrange("p (a e) -> p e a", e=E),
                    axis=X,
                )
                nc.scalar.dma_start(out=cnt[:, :], in_=c_tile[:, :])
                for a in range(A):
                    h_tile = data.tile([P, H], f32, tag="h")
                    nc.sync.dma_start(out=h_tile[:, :], in_=hs_v[a])
                    # Per-partition scalar via stride-0 broadcast: the
                    # TensorScalarPtr form can only carry one HW sync wait
                    # slot and fails codegen under Tile's two-wait pattern.
                    nc.vector.tensor_mul(
                        out=h_tile[:, :],
                        in0=h_tile[:, :],
                        in1=s_pa[:, a : a + 1].broadcast_to([P, H]),
                    )
                    nc.scalar.dma_start(out=out_v[a], in_=h_tile[:, :])
    nc.finalize()
    return nc


def _make_runner(nc, donate=True):
    """Compile nc into a cached jitted SPMD callable over the first 8 devices.

    Mirrors concourse.bass2jax.run_bass_via_pjrt's multi-core path, but keeps
    the jitted function so repeated kernel() calls don't retrace/recompile.
    Inputs/outputs are globally token-block-sharded on axis 0, so full arrays
    pass straight through with no host-side split/concat.
    """
    import jax
    import concourse.mybir as mybir
    from concourse import bass2jax
    from jax.experimental.shard_map import shard_map
    from jax.sharding import Mesh, PartitionSpec

    bass2jax.install_neuronx_cc_hook()

    partition_name = (
        nc.partition_id_tensor.name if nc.partition_id_tensor is not None else None
    )
    in_names, out_names, out_avals = [], [], []
    for alloc in nc.m.functions[0].allocations:
        if not isinstance(alloc, mybir.MemoryLocationSet):
            continue
        name = alloc.memorylocations[0].name
        if alloc.kind == "ExternalInput":
            if name != partition_name:
                in_names.append(name)
        elif alloc.kind == "ExternalOutput":
            out_names.append(name)
            out_avals.append(
                jax.core.ShapedArray(
                    tuple(alloc.tensor_shape), mybir.dt.np(alloc.dtype)
                )
            )
    n_params = len(in_names)
    all_in_names = in_names + out_names
    if partition_name is not None:
        all_in_names = all_in_names + [partition_name]
    all_in_names = tuple(all_in_names)

    def _body(*args):
        operands = list(args)
        if partition_name is not None:
            operands.append(bass2jax.partition_id_tensor())
        outs = bass2jax._bass_exec_p.bind(
            *operands,
            out_avals=tuple(out_avals),
            in_names=all_in_names,
            out_names=tuple(out_names),
            lowering_input_output_aliases=(),
            sim_require_finite=True,
            sim_require_nnan=True,
            nc=nc,
        )
        return tuple(outs)

    devices = jax.devices()[:NCORES]
    assert len(devices) == NCORES, f"need {NCORES} devices, have {len(jax.devices())}"
    mesh = Mesh(np.asarray(devices), ("core",))
    n_outs = len(out_names)
    fn = jax.jit(
        shard_map(
            _body,
            mesh=mesh,
            in_specs=(PartitionSpec("core"),) * (n_params + n_outs),
            out_specs=(PartitionSpec("core"),) * n_outs,
            check_rep=False,
        ),
        donate_argnums=tuple(range(n_params, n_params + n_outs)) if donate else (),
        keep_unused=True,
    )
    return fn, out_avals, mesh


def _get_runner():
    global _RUNNER
    if _RUNNER is None:
        _RUNNER = _make_runner(_build_nc())
    return _RUNNER


def kernel(hidden_states, probs, routing_map, top_k=2):
    hs = np.ascontiguousarray(np.asarray(hidden_states, dtype=np.float32))
    pr = np.ascontiguousarray(np.asarray(probs, dtype=np.float32))
    rt = np.ascontiguousarray(np.asarray(routing_map, dtype=np.float32))
    assert hs.shape == (T, H) and pr.shape == (T, E) and rt.shape == (T, E)

    fn, out_avals, _mesh = _get_runner()
    zeros = [
        np.zeros((NCORES * av.shape[0], *av.shape[1:]), av.dtype) for av in out_avals
    ]
    outs = fn(hs, pr, rt, *zeros)
    restored = np.asarray(outs[0])
    tokens_per_expert = (
        np.asarray(outs[1]).sum(axis=0).round().astype(np.int32)
    )
    return restored, tokens_per_expert
